# revision 18
# baseline (speedup 1.0000x reference)
"""DilateAttention (3x3 local window attention) Trainium2 Bass kernel.

Problem: q,k,v [8, 768, 56, 56] f32 -> out [8, 56, 56, 768] f32.
12 heads x head_dim 64; per-pixel softmax over a 3x3 zero-padded window.

Sharding: data-parallel over batch B=8 across the 8 NeuronCores (one image
per core). Each core computes its full image independently; host stacks.

Per-core layout ("rows on partitions"):
  partitions = [58 padded rows of head A | 58 padded rows of head B] = 116
  free       = [hd(64), w(58 padded)]  (w innermost, left-pad 1)
  row r of the image sits at partition 1+r (head A) / 59+r (head B).

k and v are stored 3-row-replicated (k3[p, j, hd, w] = k[row(p)+j-1, hd, w-1])
so every tap (di, dj) of the 3x3 window is a pure free/partition-aligned AP
view: tap product  prod = q * k3[:, 1+di, :, (1+dj):(57+dj)].

Per-pixel dot products over hd (diagonal structure, not expressible as a
dense matmul) are collapsed with an identity-weight TensorE matmul whose
PSUM output AP has a step-0 dim over hd: the has_written accumulate logic
sums the 64 hd columns in place, yielding scores [116 rows, 56 w] directly
in the softmax-friendly layout.  exp runs on ScalarE straight out of PSUM
(scores are ~N(0,1); max-subtraction is unnecessary), 1/Z is folded into
the attention weights, and the weighted-v accumulation reuses the same
identity matmul to sum the 9 taps in PSUM.
"""

import os
import sys
import threading

import numpy as np

sys.path.insert(0, "/opt/trn_rl_repo")

HEADS = 12
HD = 64
H = W = 56
B = 8
D = HEADS * HD
SCALE = HD ** (-0.5)
HP = HEADS // 2          # head pairs per core
RP = 58                  # padded rows per head block
P2 = 2 * RP              # 116 partitions
WF = 58                  # padded w (left pad 1, right pad 1)
FREE = HD * WF           # free elems per partition in k3/v3 slice

_lock = threading.Lock()
_compiled = None


def _build():
    import concourse.bacc as bacc
    import concourse.bass as bass
    import concourse.tile as tile
    import concourse.mybir as mybir
    from concourse.masks import make_identity

    f32 = mybir.dt.float32

    nc = bacc.Bacc("TRN2", target_bir_lowering=False, debug=False, num_devices=B)
    q_d = nc.dram_tensor("q", [D, H, W], f32, kind="ExternalInput").ap()
    k_d = nc.dram_tensor("k", [D, H, W], f32, kind="ExternalInput").ap()
    v_d = nc.dram_tensor("v", [D, H, W], f32, kind="ExternalInput").ap()
    o_d = nc.dram_tensor("o", [H, W, D], f32, kind="ExternalOutput").ap()
    z_d = nc.dram_tensor("zpad", [1, 3 * FREE], f32, kind="ExternalInput").ap()

    def inject0(apv, pos, n):
        # insert a step-0 (broadcast/collapse) dim into a canonical slice AP
        return bass.AP(tensor=apv.tensor, offset=apv.offset,
                       ap=[list(d) for d in apv.ap[:pos]] + [[0, n]]
                          + [list(d) for d in apv.ap[pos:]])

    with tile.TileContext(nc) as tc:
        with tc.tile_pool(name="const", bufs=1) as const_pool, \
             tc.tile_pool(name="qkv", bufs=1) as qkv_pool, \
             tc.tile_pool(name="prod", bufs=2) as prod_pool, \
             tc.tile_pool(name="small", bufs=2) as small_pool, \
             tc.tile_pool(name="outp", bufs=2) as out_pool, \
             tc.tile_pool(name="ps_s", bufs=1, space="PSUM") as ps_s, \
             tc.tile_pool(name="ps_d", bufs=1, space="PSUM") as ps_d:

            # Three identity variants: plain, and with output rows for the
            # first/last image row zeroed — used for di=-1/+1 taps so that
            # out-of-range window taps contribute exact zeros (matching the
            # reference's zero padding) without any zero-fill DMAs.
            idents = {}
            for nm, drop in (("c", ()), ("t", (1, 1 + RP)), ("b", (56, 56 + RP))):
                it = const_pool.tile([P2, P2], f32, tag=f"id_{nm}")
                make_identity(nc, it[:])
                for m in drop:
                    nc.vector.memset(it[:, m: m + 1], 0.0)
                idents[nm] = it
            ident_for_di = {-1: idents["t"], 0: idents["c"], 1: idents["b"]}
            zcol = const_pool.tile([P2, 3 * HD], f32, tag="zcol")  # zero source
            nc.vector.memset(zcol[:], 0.0)

            for hp in range(HP):
                ch = 128 * hp  # channel base of head pair

                q_sb = qkv_pool.tile([P2, HD, W], f32, tag="q_sb")
                k3 = qkv_pool.tile([P2, 3, HD, WF], f32, tag="k3")
                v3 = qkv_pool.tile([P2, 3, HD, WF], f32, tag="v3")

                # ---- loads -------------------------------------------------
                for a in range(2):  # head within pair
                    cb = ch + 64 * a
                    pb = RP * a
                    # zero never-written partitions: garbage NaN there would
                    # poison every PE-matmul output row (0*NaN=NaN in the sum)
                    for pz in (pb, pb + 57):
                        nc.sync.dma_start(out=q_sb[pz: pz + 1],
                                          in_=z_d[:, : HD * W])
                        for t3 in (k3, v3):
                            nc.sync.dma_start(out=t3[pz: pz + 1],
                                              in_=z_d[:])
                    for t3 in (k3, v3):  # out-of-range source rows
                        nc.sync.dma_start(out=t3[pb + 1: pb + 2, 0],
                                          in_=z_d[:, :FREE])
                        nc.sync.dma_start(out=t3[pb + 56: pb + 57, 2],
                                          in_=z_d[:, :FREE])
                    # q rows -> partitions 1+r
                    nc.sync.dma_start(
                        out=q_sb[pb + 1: pb + 57],
                        in_=q_d[cb: cb + 64].transpose([1, 0, 2]),
                    )
                    for t_d, t3 in ((k_d, k3), (v_d, v3)):
                        for j in range(3):
                            r0 = max(0, 1 - j)            # first valid dest row r
                            r1 = 56 + min(0, 1 - j)      # end (exclusive)
                            rows = r1 - r0
                            src = t_d[cb: cb + 64, r0 + j - 1: r0 + j - 1 + rows]
                            nc.sync.dma_start(
                                out=t3[pb + 1 + r0: pb + 1 + r0 + rows, j, :, 1: 57],
                                in_=src.transpose([1, 0, 2]),
                            )
                # w pad columns 0 and 57 for all j (zero => OOB column taps
                # contribute score 0 / value 0, matching reference padding)
                zc4 = zcol[:].rearrange("p (a b c) -> p a b c", a=3, b=HD)
                for t3 in (k3, v3):
                    for wc in (0, 57):
                        nc.vector.tensor_copy(t3[:, :, :, wc: wc + 1], zc4)

                # ---- stage A: scores --------------------------------------
                s_ps = ps_s.tile([P2, 9, W], f32, tag="s_ps")
                for t in range(9):
                    di, dj = t // 3 - 1, t % 3 - 1
                    prod = prod_pool.tile([P2, HD, W], f32, tag="prod")
                    kv = k3[:, t // 3, :, 1 + dj: 57 + dj]
                    nc.vector.tensor_mul(prod[:], q_sb[:], kv)
                    for s in range(8):  # hd slices of 8 -> N=448 <= 512 (f32)
                        out_ap = inject0(s_ps[:, t, :], 1, 8)
                        nc.tensor.matmul(
                            out_ap, ident_for_di[di][:],
                            prod[:, 8 * s: 8 * s + 8, :],
                            start=(s == 0), stop=(s == 7),
                        )

                # ---- softmax (no max-subtraction; fold 1/Z into weights) --
                e_sb = small_pool.tile([P2, 9, W], f32, tag="e_sb")
                nc.scalar.activation(
                    e_sb[:], s_ps[:], mybir.ActivationFunctionType.Exp, scale=SCALE
                )
                z = small_pool.tile([P2, W], f32, tag="z")
                nc.vector.tensor_reduce(
                    z[:], e_sb[:].transpose([0, 2, 1]),
                    mybir.AxisListType.X, mybir.AluOpType.add,
                )
                r = small_pool.tile([P2, W], f32, tag="r")
                nc.vector.reciprocal(r[:], z[:])
                eh = small_pool.tile([P2, 9, W], f32, tag="eh")
                r_b = inject0(r[:], 1, 9)
                nc.vector.tensor_mul(eh[:], e_sb[:], r_b)

                # ---- stage D: weighted v ----------------------------------
                # slice-outer loop: each PSUM slice gets its 9 accumulating
                # matmuls contiguously (interleaved accumulation groups break)
                d_ps = ps_d.tile([P2, HD, W], f32, tag="d_ps")
                for s in range(8):
                    hs = slice(8 * s, 8 * s + 8)
                    for t in range(9):
                        di, dj = t // 3 - 1, t % 3 - 1
                        dp = prod_pool.tile([P2, 8, W], f32, tag="dp")
                        vv = v3[:, t // 3, hs, 1 + dj: 57 + dj]
                        e_b = inject0(eh[:, t, :], 1, 8)
                        nc.vector.tensor_mul(dp[:], vv, e_b)
                        nc.tensor.matmul(
                            d_ps[:, hs, :], ident_for_di[di][:], dp[:],
                            start=(t == 0), stop=(t == 8),
                        )

                # ---- evacuate + store -------------------------------------
                o_sb = out_pool.tile([P2, W, HD], f32, tag="o_sb")
                nc.scalar.copy(o_sb[:].transpose([0, 2, 1]), d_ps[:])
                for a in range(2):
                    nc.sync.dma_start(
                        out=o_d[:, :, ch + 64 * a: ch + 64 * a + 64],
                        in_=o_sb[RP * a + 1: RP * a + 57],
                    )

    nc.compile()
    return nc


def _get_compiled():
    global _compiled
    with _lock:
        if _compiled is None:
            _compiled = _build()
    return _compiled


def make_in_maps(q, k, v):
    zpad = np.zeros((1, 3 * FREE), dtype=np.float32)
    return [
        {"q": np.ascontiguousarray(q[b]),
         "k": np.ascontiguousarray(k[b]),
         "v": np.ascontiguousarray(v[b]),
         "zpad": zpad}
        for b in range(B)
    ]


def kernel(q: np.ndarray, k: np.ndarray, v: np.ndarray) -> np.ndarray:
    from concourse.bass_utils import run_bass_kernel_spmd

    nc = _get_compiled()
    res = run_bass_kernel_spmd(nc, make_in_maps(q, k, v), core_ids=list(range(B)))
    return np.stack([res.results[b]["o"] for b in range(B)], axis=0)
